# revision 48
# baseline (speedup 1.0000x reference)
"""Multi-head causal attention on 8 Trainium2 NeuronCores.

Sharding: core c -> batch b = c // 4, head group g = c % 4 (4 of 16 heads).
Each core computes q/k/v for its 4 heads, causal softmax attention, and a
partial output  z_norm @ W_O[heads]  of shape [S, D] (fp16).  Host sums the
4 head-group partials per batch (f32) and adds b_O.

Fully SBUF-resident fp16 pipeline (no DRAM scratch round-trip):
  - All inputs cast to fp16 on host; fp16 matmuls run at 1 cycle/row and
    keep ~8x error margin under the 2e-2 gate (measured rel err ~6e-4).
  - Bootstrap: heads 0+1 q/k plus head 0's v run in x-column order so the
    PE starts dense as soon as w0 + column 0 land (a trickled start would
    keep resetting the tensor engine's p-state ramp).
  - Per head: scoresT = kT.T@q chunks with the diagonal 512-chunk's dead
    half skipped, additive causal mask on the diagonal 128-block, exp on
    ACT staggered one chunk ahead of PV; the next head's q/k/v projections
    are drained as PE filler inside the scores phases so the PE never
    waits on ACT.
  - v is computed in natural [seq, e] layout (xT-stationary matmuls) with
    a ones-column appended, so the PV psum's column E accumulates softmax
    denominators for free; z is normalized on DVE and PE-transposed into
    zT (transposes deferred one i-tile so they never wait on the DVE).
  - phase C accumulates all 4 heads from SBUF, stages a full 2048-wide
    fp16 row, and ships it as one contiguous DMA per seq tile.
Measured: ~343 us/core HW exec (baseline: 433 us); PE issue floor for this
structure is ~292 us + ~30 us input-DMA fill + ~14 us framework pre/post.
"""

import sys

for _p in ("/opt/trn_rl_repo",):
    if _p not in sys.path:
        sys.path.insert(0, _p)

import numpy as np

import concourse.bass as bass
from concourse import bacc
import concourse.mybir as mybir
import concourse.tile as tile
from concourse.bass_utils import run_bass_kernel_spmd

F32 = mybir.dt.float32
F16 = mybir.dt.float16

B, S, D, H, E = 2, 2048, 2048, 16, 128
HL = 4          # heads per core
NCORES = 8
P = 128         # partitions
CH = 512        # free-dim chunk
S_T = S // P    # 16 seq tiles
S_C = S // CH   # 4 seq chunks
D_T = D // P    # 16 model-dim subtiles
D_C = D // CH   # 4 model-dim chunks
INV_SQRT_E = 1.0 / float(np.sqrt(E))


def _trace_kernel(tc, xt, wqkv, wo, bqk, bvb, ident, dmsk, outp):
    nc = tc.nc
    ts = bass.ts

    xt3 = xt.rearrange("(o p) s -> p o s", p=P)          # [128, 16, 2048]
    # wqkv host layout: [HL, D, 3E] -> [128, 4, 16, 384]
    w4 = wqkv.rearrange("(h o p) e -> p h o e", h=HL, p=P)
    wo3 = wo.rearrange("(h p) d -> p h d", p=P)          # [128, 4, 2048]
    bqk2 = bqk.rearrange("m (h p) -> p m h", p=P)        # [128, 2, 4]
    out3 = outp.rearrange("(t p) d -> t p d", p=P)       # [16, 128, 2048]

    from contextlib import ExitStack

    with ExitStack() as top:
        const_pool = top.enter_context(tc.tile_pool(name="consts", bufs=1))
        # xt and wo share one single-slot pool: wo (needed only in phase C)
        # reuses xt's SBUF after the last v/q/k projection consumed it
        xt_pool = top.enter_context(tc.tile_pool(name="xtp", bufs=1))
        w_pool = top.enter_context(tc.tile_pool(name="wp", bufs=2))
        qk_pool = top.enter_context(tc.tile_pool(name="qkp", bufs=4))
        va_pool = top.enter_context(tc.tile_pool(name="vap", bufs=2))
        z_pool = top.enter_context(tc.tile_pool(name="zp", bufs=1))
        zsb_pool = top.enter_context(tc.tile_pool(name="zsbp", bufs=3))
        e_pool = top.enter_context(tc.tile_pool(name="ep", bufs=2))
        stage_pool = top.enter_context(tc.tile_pool(name="stp", bufs=4))
        small_pool = top.enter_context(tc.tile_pool(name="smallp", bufs=4))

        psA = top.enter_context(tc.tile_pool(name="psA", bufs=2, space="PSUM"))
        psS = top.enter_context(tc.tile_pool(name="psS", bufs=3, space="PSUM"))
        psZ = top.enter_context(tc.tile_pool(name="psZ", bufs=2, space="PSUM"))
        psT = top.enter_context(tc.tile_pool(name="psT", bufs=1, space="PSUM"))

        # ---------------- constants (all host-precomputed DMAs; no Q7
        # affine_select work on the critical startup path) ----------------
        identity = const_pool.tile([P, P], F16)
        nc.gpsimd.dma_start(identity, ident[:, :])
        # additive causal mask for the 128-wide diagonal block of scoresT:
        # partition = j (key), col = i (query); valid iff col >= partition
        dmask = const_pool.tile([P, P], F32)
        nc.gpsimd.dma_start(dmask, dmsk[:, :])
        bias_qk = const_pool.tile([P, 2, HL], F32)
        nc.gpsimd.dma_start(bias_qk, bqk2)
        # b_V broadcast along partitions (host-tiled)
        bias_v = const_pool.tile([P, HL * E], F16)
        nc.gpsimd.dma_start(bias_v, bvb[:, :])

        # warm the ACT engine's Exp table while phase A runs so the first
        # real exp doesn't pay the table-load stall
        warm = const_pool.tile([P, 1], F16)
        nc.scalar.activation(warm, identity[:, :1], mybir.ActivationFunctionType.Exp)

        # ---------------- bulk input DMA ----------------
        # xt split across two DMA queues (sync + scalar) for 2x fill rate;
        # head-0 weights on the gpsimd queue in d-order so the first q psum
        # group starts as soon as possible.  Later heads' weights and wo
        # also go on gpsimd so their WAR-gated prefetches can never block
        # the sync queue that carries phase C's output tiles.
        w_sb = [None] * HL
        w_sb[0] = w_pool.tile([P, D_T, 3 * E], F16, name="w_h")
        xt_sb = xt_pool.tile([P, D_T, S], F16)
        # xt as 16 full-row DMAs (4KB/partition contiguous) spread over four
        # DMA queues: the ~0.7us/trigger descriptor-generation rate is the
        # startup bottleneck, so fewer+bigger contiguous transfers on more
        # rings fill SBUF fastest.
        # both hardware DMA rings deliver strictly in consumption order:
        # w0 and column 0 first (interleaved per-d so the first q psum chain
        # starts immediately), then columns 1-3; even d on sync, odd on
        # scalar so the two rings split every column's bytes evenly.
        # w0 split in two contiguous halves so both rings carry equal bytes
        # before column 0 completes (the dense-start gate)
        HD = D_T // 2
        nc.sync.dma_start(w_sb[0][:, :HD, :], w4[:, 0, :HD, :])
        nc.scalar.dma_start(w_sb[0][:, HD:, :], w4[:, 0, HD:, :])
        for c in range(S_C):
            for d in range(D_T):
                eng = nc.sync if d % 2 == 0 else nc.scalar
                eng.dma_start(xt_sb[:, d, ts(c, CH)], xt3[:, d, ts(c, CH)])
        for h in range(1, HL):
            w_sb[h] = w_pool.tile([P, D_T, 3 * E], F16, name="w_h")
            nc.gpsimd.dma_start(w_sb[h], w4[:, h])

        wo_sb = xt_pool.tile([P, HL, D], F16, name="wo_sb")
        nc.gpsimd.dma_start(wo_sb, wo3)

        zT = z_pool.tile([P, HL, S_T, P], F16)

        def emit_proj_chunk(h, m, c, dst, rev=False):
            """one 512-wide q/k projection psum group (+bias, q pre-scaled).
            rev runs the d accumulation backwards: the very first group uses
            it so its first matmul waits for the last-arriving DMA tile and
            the PE then runs dense (a trickled start would keep resetting
            the tensor engine's p-state ramp)."""
            w_h = w_sb[h]
            ps = psA.tile([P, CH], F32, name="ps_mm")
            order = range(D_T - 1, -1, -1) if rev else range(D_T)
            for d in order:
                nc.tensor.matmul(
                    ps,
                    w_h[:, d, ts(m, E)],
                    xt_sb[:, d, ts(c, CH)],
                    start=(d == (D_T - 1 if rev else 0)),
                    stop=(d == (0 if rev else D_T - 1)),
                )
            # q: bq pre-scaled by 1/sqrt(E) on host
            nc.vector.tensor_scalar(
                dst[:, ts(c, CH)], ps,
                INV_SQRT_E if m == 0 else 1.0,
                bias_qk[:, m, h, None],
                op0=mybir.AluOpType.mult,
                op1=mybir.AluOpType.add,
            )

        def emit_v_tile(h, jt, v_aug):
            """v j-tile in natural layout: pure PE work with no cross-engine
            deps -- interleaved into the scores phases as PE gap filler."""
            w_h = w_sb[h]
            ps = psA.tile([P, P], F32, name="ps_mm")
            for d in range(D_T):
                nc.tensor.matmul(
                    ps,
                    xt_sb[:, d, ts(jt, P)],
                    w_h[:, d, ts(2, E)],
                    start=(d == 0),
                    stop=(d == D_T - 1),
                )
            nc.vector.tensor_add(v_aug[:, jt, :E], ps, bias_v[:, ts(h, E)])

        def alloc_qk():
            qT = qk_pool.tile([P, S], F16, name="qT")
            kT = qk_pool.tile([P, S], F16, name="kT")
            return [qT, kT]

        def alloc_va():
            v_aug = va_pool.tile([P, S_T, E + 1], F16, name="v_aug")
            nc.vector.memset(v_aug[:, :, E : E + 1], 1.0)
            return v_aug

        def qk_units(h, qT, kT):
            units = []
            for c in range(S_C):
                units.append(lambda c_=c: emit_proj_chunk(h, 0, c_, qT))
                units.append(lambda c_=c: emit_proj_chunk(h, 1, c_, kT))
            return units

        def v_units(h, v_aug):
            return [
                (lambda jt_=jt: emit_v_tile(h, jt_, v_aug)) for jt in range(S_T)
            ]

        def emit_scores(h, c, qT, kT, extra):
            """scoresT (kT stationary, qT moving) + mask + exp for i-chunk c.
            `extra` thunks (v-tile emitters) are drained one per score tile
            so the PE keeps working while ACT chews through the exps."""
            n_jt = S_C * c + S_C
            expT = e_pool.tile(
                [P, n_jt, CH], F16, name="expT", tag=f"expT{c % 2}", bufs=1
            )
            for jt in range(n_jt):
                sps = psS.tile([P, CH], F32, name="sps")
                b = jt - S_C * c
                if b > 0:
                    # diagonal chunk: columns < b*128 are causally dead --
                    # compute only the valid suffix
                    nc.tensor.matmul(
                        sps[:, b * P :], kT[:, ts(jt, P)],
                        qT[:, c * CH + b * P : (c + 1) * CH],
                        start=True, stop=True,
                    )
                else:
                    nc.tensor.matmul(
                        sps, kT[:, ts(jt, P)], qT[:, ts(c, CH)],
                        start=True, stop=True,
                    )
                if b >= 0:
                    nc.vector.tensor_add(sps[:, ts(b, P)], sps[:, ts(b, P)], dmask)
                    nc.scalar.activation(
                        expT[:, jt, b * P :], sps[:, b * P :],
                        mybir.ActivationFunctionType.Exp,
                    )
                else:
                    nc.scalar.activation(
                        expT[:, jt, :], sps, mybir.ActivationFunctionType.Exp,
                    )
                if extra:
                    extra.pop(0)()
            return expT

        pending_z = []

        def flush_z(h, limit):
            while len(pending_z) > limit:
                i, z_sb = pending_z.pop(0)
                tp = psT.tile([P, P], F16, name="tp")
                nc.tensor.transpose(tp, z_sb, identity)
                nc.vector.tensor_copy(zT[:, h, i, :], tp)

        def emit_pv(h, c, expT, v_aug):
            """PV + row-sum normalize for the 4 i-tiles of chunk c; the PE
            transpose of tile i is deferred until after tile i+1's matmuls
            so it never waits on the DVE normalize."""
            for a in range(S_C):
                i = S_C * c + a
                z_ps = psZ.tile([P, E + 1], F32, name="z_ps")
                for jt in range(i + 1):
                    nc.tensor.matmul(
                        z_ps,
                        expT[:, jt, ts(a, P)],
                        v_aug[:, jt, :],
                        start=(jt == 0),
                        stop=(jt == i),
                    )
                rec = small_pool.tile([P, 1], F32, name="rec")
                nc.vector.reciprocal(rec, z_ps[:, E : E + 1])
                z_sb = zsb_pool.tile([P, E], F16, name="z_sb")
                nc.vector.tensor_scalar_mul(z_sb, z_ps[:, :E], rec)
                pending_z.append((i, z_sb))
                flush_z(h, 1)

        # ---------------- Phase C emission units ----------------
        # psS (bufs=3) rotates with the scores psums; alternate the
        # psum->stage copies between ACT and DVE to halve per-tile latency.
        # A whole 2048-wide row stages into one tile and ships as a single
        # contiguous DMA (4KB/partition) -- far cheaper on the sync ring
        # than 4 chunked transfers.
        def emit_c_unit(t):
            ot = stage_pool.tile([P, D], F16, name="ot")
            for dc in range(D_C):
                ops = psS.tile([P, CH], F32, name="sps")
                for lh in range(HL):
                    nc.tensor.matmul(
                        ops,
                        zT[:, lh, t, :],
                        wo_sb[:, lh, ts(dc, CH)],
                        start=(lh == 0),
                        stop=(lh == HL - 1),
                    )
                if dc % 2 == 0:
                    nc.scalar.copy(ot[:, ts(dc, CH)], ops)
                else:
                    nc.vector.tensor_copy(ot[:, ts(dc, CH)], ops)
            nc.sync.dma_start(out3[t], ot)

        # ---------------- fused A/B/C pipeline over heads ----------------
        # Bootstrap: heads 0+1 q/k and head 0's v run in x-column order
        # (~17us of PE work per column comfortably covers each column's DMA
        # arrival).  Each head's scores phases then drain the next head's
        # remaining phase-A work as filler so the PE never waits on ACT:
        #   B(0): v(1),  B(1): q/k(2) + v(2),  B(2): q/k(3) + v(3),
        #   B(3): early phase-C tiles (only those whose zT rows are ready).
        heads = [alloc_qk() + [alloc_va()], alloc_qk() + [alloc_va()]]
        for c in range(S_C):
            emit_proj_chunk(0, 0, c, heads[0][0], rev=(c == 0))
            emit_proj_chunk(0, 1, c, heads[0][1])
            for jt in range(S_C * c, S_C * c + S_C):
                emit_v_tile(0, jt, heads[0][2])
            emit_proj_chunk(1, 0, c, heads[1][0])
            emit_proj_chunk(1, 1, c, heads[1][1])
        c_units = [(lambda t_=t: emit_c_unit(t_)) for t in range(S_T)]
        for h in range(HL):
            qT, kT, v_aug = heads[h]
            fill_s2 = fill_s3 = nxt = []
            if h + 1 < HL:
                if h + 1 >= len(heads):
                    # lazy alloc: the 4-slot qk and 2-slot v_aug rotations
                    # free head h-1's tiles, whose last readers completed
                    # during head h-1's own phases
                    heads.append(alloc_qk() + [alloc_va()])
                    nv = v_units(h + 1, heads[h + 1][2])
                    qk = qk_units(h + 1, heads[h + 1][0], heads[h + 1][1])
                    # interleave: one qk chunk-group per two v tiles
                    nxt = [u for pair in zip(nv[0::2], nv[1::2], qk) for u in pair]
                else:
                    nxt = v_units(h + 1, heads[h + 1][2])
                fill_s2 = fill_s3 = nxt
            # head 3 gets no fillers: phase C drains contiguously after it
            expT = [None] * S_C
            expT[0] = emit_scores(h, 0, qT, kT, nxt)
            expT[1] = emit_scores(h, 1, qT, kT, nxt)
            emit_pv(h, 0, expT[0], v_aug)
            expT[2] = emit_scores(h, 2, qT, kT, fill_s2)
            emit_pv(h, 1, expT[1], v_aug)
            expT[3] = emit_scores(h, 3, qT, kT, fill_s3)
            emit_pv(h, 2, expT[2], v_aug)
            emit_pv(h, 3, expT[3], v_aug)
            for lst in (nxt, fill_s2, fill_s3):
                while lst:
                    lst.pop(0)()
            flush_z(h, 0)

        for u in c_units:
            u()


_NC_CACHE = {}
LAST_RESULTS = None


def _get_nc():
    if "nc" not in _NC_CACHE:
        nc = bacc.Bacc("TRN2", target_bir_lowering=False, debug=False)
        xt = nc.dram_tensor("xt", [D, S], F16, kind="ExternalInput")
        wqkv = nc.dram_tensor("wqkv", [HL * D, 3 * E], F16, kind="ExternalInput")
        wo = nc.dram_tensor("wo", [HL * E, D], F16, kind="ExternalInput")
        bqk = nc.dram_tensor("bqk", [2, HL * E], F32, kind="ExternalInput")
        bvb = nc.dram_tensor("bvb", [P, HL * E], F32, kind="ExternalInput")
        ident = nc.dram_tensor("ident", [P, P], F16, kind="ExternalInput")
        dmsk = nc.dram_tensor("dmsk", [P, P], F32, kind="ExternalInput")
        outp = nc.dram_tensor("outp", [S, D], F16, kind="ExternalOutput")
        with tile.TileContext(nc) as tc:
            _trace_kernel(tc, xt, wqkv, wo, bqk, bvb, ident, dmsk, outp)
        nc.compile()
        _NC_CACHE["nc"] = nc
    return _NC_CACHE["nc"]


def kernel(normalized_resid_pre, W_Q, W_K, W_V, W_O, b_Q, b_K, b_V, b_O):
    x = np.asarray(normalized_resid_pre, np.float32)
    W_Q = np.asarray(W_Q, np.float32)
    W_K = np.asarray(W_K, np.float32)
    W_V = np.asarray(W_V, np.float32)
    W_O = np.asarray(W_O, np.float32)
    b_Q = np.asarray(b_Q, np.float32)
    b_K = np.asarray(b_K, np.float32)
    b_V = np.asarray(b_V, np.float32)
    b_O = np.asarray(b_O, np.float32)

    nc = _get_nc()

    # wqkv[h] = [W_Q[h] | W_K[h] | W_V[h]] along the E axis -> [HL*D, 3E]
    xt16 = [np.ascontiguousarray(x[b].T.astype(np.float16)) for b in range(B)]
    wqkv16 = []
    wo16 = []
    bqk32 = []
    bvb32 = []
    for g in range(NCORES // B):
        hs = slice(g * HL, (g + 1) * HL)
        wqkv16.append(
            np.ascontiguousarray(
                np.concatenate([W_Q[hs], W_K[hs], W_V[hs]], axis=2)
                .reshape(HL * D, 3 * E)
                .astype(np.float16)
            )
        )
        wo16.append(np.ascontiguousarray(W_O[hs].reshape(HL * E, D).astype(np.float16)))
        bqk32.append(
            np.ascontiguousarray(
                np.stack(
                    [
                        b_Q[hs].reshape(-1) * np.float32(INV_SQRT_E),
                        b_K[hs].reshape(-1),
                    ]
                )
            )
        )
        bvb32.append(
            np.ascontiguousarray(np.tile(b_V[hs].reshape(1, -1), (P, 1)))
        )

    ident16 = np.eye(P, dtype=np.float16)
    col = np.arange(P, dtype=np.float32)
    dmsk32 = np.ascontiguousarray(
        np.where(col[None, :] >= col[:, None], np.float32(0), np.float32(-1e9))
    )

    in_maps = []
    for core in range(NCORES):
        b, g = core // (NCORES // B), core % (NCORES // B)
        in_maps.append(
            {
                "xt": xt16[b],
                "wqkv": wqkv16[g],
                "wo": wo16[g],
                "bqk": bqk32[g],
                "bvb": bvb32[g],
                "ident": ident16,
                "dmsk": dmsk32,
            }
        )

    res = run_bass_kernel_spmd(nc, in_maps, core_ids=list(range(NCORES)))
    global LAST_RESULTS
    LAST_RESULTS = res
    out = np.zeros((B, S, D), np.float32)
    for core in range(NCORES):
        out[core // (NCORES // B)] += res.results[core]["outp"].astype(np.float32)
    out += b_O[None, None, :]
    return out


# revision 49
# speedup vs baseline: 1.0021x; 1.0021x over previous
"""Multi-head causal attention on 8 Trainium2 NeuronCores.

Sharding: core c -> batch b = c // 4, head group g = c % 4 (4 of 16 heads).
Each core computes q/k/v for its 4 heads, causal softmax attention, and a
partial output  z_norm @ W_O[heads]  of shape [S, D] (fp16).  Host sums the
4 head-group partials per batch (f32) and adds b_O.

Fully SBUF-resident fp16 pipeline (no DRAM scratch round-trip):
  - All inputs cast to fp16 on host; fp16 matmuls run at 1 cycle/row and
    keep ~8x error margin under the 2e-2 gate (measured rel err ~6e-4).
  - Bootstrap: heads 0+1 q/k plus head 0's v run in x-column order so the
    PE starts dense as soon as w0 + column 0 land (a trickled start would
    keep resetting the tensor engine's p-state ramp).
  - Per head: scoresT = kT.T@q chunks with the diagonal 512-chunk's dead
    half skipped, additive causal mask on the diagonal 128-block, exp on
    ACT staggered one chunk ahead of PV; the next head's q/k/v projections
    are drained as PE filler inside the scores phases so the PE never
    waits on ACT.
  - v is computed in natural [seq, e] layout (xT-stationary matmuls) with
    a ones-column appended, so the PV psum's column E accumulates softmax
    denominators for free; z is normalized on DVE and PE-transposed into
    zT (transposes deferred one i-tile so they never wait on the DVE).
  - phase C accumulates all 4 heads from SBUF, stages a full 2048-wide
    fp16 row, and ships it as one contiguous DMA per seq tile.
Measured: ~343 us/core HW exec (baseline: 433 us); PE issue floor for this
structure is ~292 us + ~30 us input-DMA fill + ~14 us framework pre/post.
"""

import sys

for _p in ("/opt/trn_rl_repo",):
    if _p not in sys.path:
        sys.path.insert(0, _p)

import numpy as np

import concourse.bass as bass
from concourse import bacc
import concourse.mybir as mybir
import concourse.tile as tile
from concourse.bass_utils import run_bass_kernel_spmd

F32 = mybir.dt.float32
F16 = mybir.dt.float16

B, S, D, H, E = 2, 2048, 2048, 16, 128
HL = 4          # heads per core
NCORES = 8
P = 128         # partitions
CH = 512        # free-dim chunk
S_T = S // P    # 16 seq tiles
S_C = S // CH   # 4 seq chunks
D_T = D // P    # 16 model-dim subtiles
D_C = D // CH   # 4 model-dim chunks
INV_SQRT_E = 1.0 / float(np.sqrt(E))


def _trace_kernel(tc, xt, wqkv, wo, bqk, bvb, ident, dmsk, outp):
    nc = tc.nc
    ts = bass.ts

    xt3 = xt.rearrange("(o p) s -> p o s", p=P)          # [128, 16, 2048]
    # wqkv host layout: [HL, D, 3E] -> [128, 4, 16, 384]
    w4 = wqkv.rearrange("(h o p) e -> p h o e", h=HL, p=P)
    wo3 = wo.rearrange("(h p) d -> p h d", p=P)          # [128, 4, 2048]
    bqk2 = bqk.rearrange("m (h p) -> p m h", p=P)        # [128, 2, 4]
    out3 = outp.rearrange("(t p) d -> t p d", p=P)       # [16, 128, 2048]

    from contextlib import ExitStack

    with ExitStack() as top:
        const_pool = top.enter_context(tc.tile_pool(name="consts", bufs=1))
        # xt and wo share one single-slot pool: wo (needed only in phase C)
        # reuses xt's SBUF after the last v/q/k projection consumed it
        xt_pool = top.enter_context(tc.tile_pool(name="xtp", bufs=1))
        w_pool = top.enter_context(tc.tile_pool(name="wp", bufs=2))
        qk_pool = top.enter_context(tc.tile_pool(name="qkp", bufs=4))
        va_pool = top.enter_context(tc.tile_pool(name="vap", bufs=2))
        z_pool = top.enter_context(tc.tile_pool(name="zp", bufs=1))
        zsb_pool = top.enter_context(tc.tile_pool(name="zsbp", bufs=3))
        e_pool = top.enter_context(tc.tile_pool(name="ep", bufs=2))
        stage_pool = top.enter_context(tc.tile_pool(name="stp", bufs=4))
        small_pool = top.enter_context(tc.tile_pool(name="smallp", bufs=4))

        psA = top.enter_context(tc.tile_pool(name="psA", bufs=2, space="PSUM"))
        psS = top.enter_context(tc.tile_pool(name="psS", bufs=3, space="PSUM"))
        psZ = top.enter_context(tc.tile_pool(name="psZ", bufs=2, space="PSUM"))
        psT = top.enter_context(tc.tile_pool(name="psT", bufs=1, space="PSUM"))

        # ---------------- constants (all host-precomputed DMAs; no Q7
        # affine_select work on the critical startup path) ----------------
        identity = const_pool.tile([P, P], F16)
        nc.gpsimd.dma_start(identity, ident[:, :])
        # additive causal mask for the 128-wide diagonal block of scoresT:
        # partition = j (key), col = i (query); valid iff col >= partition
        dmask = const_pool.tile([P, P], F32)
        nc.gpsimd.dma_start(dmask, dmsk[:, :])
        bias_qk = const_pool.tile([P, 2, HL], F32)
        nc.gpsimd.dma_start(bias_qk, bqk2)
        # b_V broadcast along partitions (host-tiled)
        bias_v = const_pool.tile([P, HL * E], F16)
        nc.gpsimd.dma_start(bias_v, bvb[:, :])

        # warm the ACT engine's Exp table while phase A runs so the first
        # real exp doesn't pay the table-load stall
        warm = const_pool.tile([P, 1], F16)
        nc.scalar.activation(warm, identity[:, :1], mybir.ActivationFunctionType.Exp)

        # ---------------- bulk input DMA ----------------
        # xt split across two DMA queues (sync + scalar) for 2x fill rate;
        # head-0 weights on the gpsimd queue in d-order so the first q psum
        # group starts as soon as possible.  Later heads' weights and wo
        # also go on gpsimd so their WAR-gated prefetches can never block
        # the sync queue that carries phase C's output tiles.
        w_sb = [None] * HL
        w_sb[0] = w_pool.tile([P, D_T, 3 * E], F16, name="w_h")
        xt_sb = xt_pool.tile([P, D_T, S], F16)
        # xt as 16 full-row DMAs (4KB/partition contiguous) spread over four
        # DMA queues: the ~0.7us/trigger descriptor-generation rate is the
        # startup bottleneck, so fewer+bigger contiguous transfers on more
        # rings fill SBUF fastest.
        # both hardware DMA rings deliver strictly in consumption order:
        # w0 and column 0 first (interleaved per-d so the first q psum chain
        # starts immediately), then columns 1-3; even d on sync, odd on
        # scalar so the two rings split every column's bytes evenly.
        nc.sync.dma_start(w_sb[0], w4[:, 0])
        for c in range(S_C):
            for d in range(D_T):
                eng = nc.sync if d % 2 == 0 else nc.scalar
                eng.dma_start(xt_sb[:, d, ts(c, CH)], xt3[:, d, ts(c, CH)])
        for h in range(1, HL):
            w_sb[h] = w_pool.tile([P, D_T, 3 * E], F16, name="w_h")
            nc.gpsimd.dma_start(w_sb[h], w4[:, h])

        wo_sb = xt_pool.tile([P, HL, D], F16, name="wo_sb")
        nc.gpsimd.dma_start(wo_sb, wo3)

        zT = z_pool.tile([P, HL, S_T, P], F16)

        def emit_proj_chunk(h, m, c, dst, rev=False):
            """one 512-wide q/k projection psum group (+bias, q pre-scaled).
            rev runs the d accumulation backwards: the very first group uses
            it so its first matmul waits for the last-arriving DMA tile and
            the PE then runs dense (a trickled start would keep resetting
            the tensor engine's p-state ramp)."""
            w_h = w_sb[h]
            ps = psA.tile([P, CH], F32, name="ps_mm")
            order = range(D_T - 1, -1, -1) if rev else range(D_T)
            for d in order:
                nc.tensor.matmul(
                    ps,
                    w_h[:, d, ts(m, E)],
                    xt_sb[:, d, ts(c, CH)],
                    start=(d == (D_T - 1 if rev else 0)),
                    stop=(d == (0 if rev else D_T - 1)),
                )
            # q: bq pre-scaled by 1/sqrt(E) on host
            nc.vector.tensor_scalar(
                dst[:, ts(c, CH)], ps,
                INV_SQRT_E if m == 0 else 1.0,
                bias_qk[:, m, h, None],
                op0=mybir.AluOpType.mult,
                op1=mybir.AluOpType.add,
            )

        def emit_v_tile(h, jt, v_aug):
            """v j-tile in natural layout: pure PE work with no cross-engine
            deps -- interleaved into the scores phases as PE gap filler."""
            w_h = w_sb[h]
            ps = psA.tile([P, P], F32, name="ps_mm")
            for d in range(D_T):
                nc.tensor.matmul(
                    ps,
                    xt_sb[:, d, ts(jt, P)],
                    w_h[:, d, ts(2, E)],
                    start=(d == 0),
                    stop=(d == D_T - 1),
                )
            nc.vector.tensor_add(v_aug[:, jt, :E], ps, bias_v[:, ts(h, E)])

        def alloc_qk():
            qT = qk_pool.tile([P, S], F16, name="qT")
            kT = qk_pool.tile([P, S], F16, name="kT")
            return [qT, kT]

        def alloc_va():
            v_aug = va_pool.tile([P, S_T, E + 1], F16, name="v_aug")
            nc.vector.memset(v_aug[:, :, E : E + 1], 1.0)
            return v_aug

        def qk_units(h, qT, kT):
            units = []
            for c in range(S_C):
                units.append(lambda c_=c: emit_proj_chunk(h, 0, c_, qT))
                units.append(lambda c_=c: emit_proj_chunk(h, 1, c_, kT))
            return units

        def v_units(h, v_aug):
            return [
                (lambda jt_=jt: emit_v_tile(h, jt_, v_aug)) for jt in range(S_T)
            ]

        def emit_scores(h, c, qT, kT, extra):
            """scoresT (kT stationary, qT moving) + mask + exp for i-chunk c.
            `extra` thunks (v-tile emitters) are drained one per score tile
            so the PE keeps working while ACT chews through the exps."""
            n_jt = S_C * c + S_C
            expT = e_pool.tile(
                [P, n_jt, CH], F16, name="expT", tag=f"expT{c % 2}", bufs=1
            )
            for jt in range(n_jt):
                sps = psS.tile([P, CH], F32, name="sps")
                b = jt - S_C * c
                if b > 0:
                    # diagonal chunk: columns < b*128 are causally dead --
                    # compute only the valid suffix
                    nc.tensor.matmul(
                        sps[:, b * P :], kT[:, ts(jt, P)],
                        qT[:, c * CH + b * P : (c + 1) * CH],
                        start=True, stop=True,
                    )
                else:
                    nc.tensor.matmul(
                        sps, kT[:, ts(jt, P)], qT[:, ts(c, CH)],
                        start=True, stop=True,
                    )
                if b >= 0:
                    nc.vector.tensor_add(sps[:, ts(b, P)], sps[:, ts(b, P)], dmask)
                    nc.scalar.activation(
                        expT[:, jt, b * P :], sps[:, b * P :],
                        mybir.ActivationFunctionType.Exp,
                    )
                else:
                    nc.scalar.activation(
                        expT[:, jt, :], sps, mybir.ActivationFunctionType.Exp,
                    )
                if extra:
                    extra.pop(0)()
            return expT

        pending_z = []

        def flush_z(h, limit):
            while len(pending_z) > limit:
                i, z_sb = pending_z.pop(0)
                tp = psT.tile([P, P], F16, name="tp")
                nc.tensor.transpose(tp, z_sb, identity)
                nc.vector.tensor_copy(zT[:, h, i, :], tp)

        def emit_pv(h, c, expT, v_aug):
            """PV + row-sum normalize for the 4 i-tiles of chunk c; the PE
            transpose of tile i is deferred until after tile i+1's matmuls
            so it never waits on the DVE normalize."""
            for a in range(S_C):
                i = S_C * c + a
                z_ps = psZ.tile([P, E + 1], F32, name="z_ps")
                for jt in range(i + 1):
                    nc.tensor.matmul(
                        z_ps,
                        expT[:, jt, ts(a, P)],
                        v_aug[:, jt, :],
                        start=(jt == 0),
                        stop=(jt == i),
                    )
                rec = small_pool.tile([P, 1], F32, name="rec")
                nc.vector.reciprocal(rec, z_ps[:, E : E + 1])
                z_sb = zsb_pool.tile([P, E], F16, name="z_sb")
                nc.vector.tensor_scalar_mul(z_sb, z_ps[:, :E], rec)
                pending_z.append((i, z_sb))
                flush_z(h, 1)

        # ---------------- Phase C emission units ----------------
        # psS (bufs=3) rotates with the scores psums; alternate the
        # psum->stage copies between ACT and DVE to halve per-tile latency.
        # A whole 2048-wide row stages into one tile and ships as a single
        # contiguous DMA (4KB/partition) -- far cheaper on the sync ring
        # than 4 chunked transfers.
        def emit_c_unit(t):
            ot = stage_pool.tile([P, D], F16, name="ot")
            for dc in range(D_C):
                ops = psS.tile([P, CH], F32, name="sps")
                for lh in range(HL):
                    nc.tensor.matmul(
                        ops,
                        zT[:, lh, t, :],
                        wo_sb[:, lh, ts(dc, CH)],
                        start=(lh == 0),
                        stop=(lh == HL - 1),
                    )
                if dc % 2 == 0:
                    nc.scalar.copy(ot[:, ts(dc, CH)], ops)
                else:
                    nc.vector.tensor_copy(ot[:, ts(dc, CH)], ops)
            nc.sync.dma_start(out3[t], ot)

        # ---------------- fused A/B/C pipeline over heads ----------------
        # Bootstrap: heads 0+1 q/k and head 0's v run in x-column order
        # (~17us of PE work per column comfortably covers each column's DMA
        # arrival).  Each head's scores phases then drain the next head's
        # remaining phase-A work as filler so the PE never waits on ACT:
        #   B(0): v(1),  B(1): q/k(2) + v(2),  B(2): q/k(3) + v(3),
        #   B(3): early phase-C tiles (only those whose zT rows are ready).
        heads = [alloc_qk() + [alloc_va()], alloc_qk() + [alloc_va()]]
        for c in range(S_C):
            emit_proj_chunk(0, 0, c, heads[0][0], rev=(c == 0))
            emit_proj_chunk(0, 1, c, heads[0][1])
            for jt in range(S_C * c, S_C * c + S_C):
                emit_v_tile(0, jt, heads[0][2])
            emit_proj_chunk(1, 0, c, heads[1][0])
            emit_proj_chunk(1, 1, c, heads[1][1])
        c_units = [(lambda t_=t: emit_c_unit(t_)) for t in range(S_T)]
        for h in range(HL):
            qT, kT, v_aug = heads[h]
            fill_s2 = fill_s3 = nxt = []
            if h + 1 < HL:
                if h + 1 >= len(heads):
                    # lazy alloc: the 4-slot qk and 2-slot v_aug rotations
                    # free head h-1's tiles, whose last readers completed
                    # during head h-1's own phases
                    heads.append(alloc_qk() + [alloc_va()])
                    nv = v_units(h + 1, heads[h + 1][2])
                    qk = qk_units(h + 1, heads[h + 1][0], heads[h + 1][1])
                    # interleave: one qk chunk-group per two v tiles
                    nxt = [u for pair in zip(nv[0::2], nv[1::2], qk) for u in pair]
                else:
                    nxt = v_units(h + 1, heads[h + 1][2])
                fill_s2 = fill_s3 = nxt
            # head 3 gets no fillers: phase C drains contiguously after it
            expT = [None] * S_C
            expT[0] = emit_scores(h, 0, qT, kT, nxt)
            expT[1] = emit_scores(h, 1, qT, kT, nxt)
            emit_pv(h, 0, expT[0], v_aug)
            expT[2] = emit_scores(h, 2, qT, kT, fill_s2)
            emit_pv(h, 1, expT[1], v_aug)
            expT[3] = emit_scores(h, 3, qT, kT, fill_s3)
            emit_pv(h, 2, expT[2], v_aug)
            emit_pv(h, 3, expT[3], v_aug)
            for lst in (nxt, fill_s2, fill_s3):
                while lst:
                    lst.pop(0)()
            flush_z(h, 0)

        for u in c_units:
            u()


_NC_CACHE = {}
LAST_RESULTS = None


def _get_nc():
    if "nc" not in _NC_CACHE:
        nc = bacc.Bacc("TRN2", target_bir_lowering=False, debug=False)
        xt = nc.dram_tensor("xt", [D, S], F16, kind="ExternalInput")
        wqkv = nc.dram_tensor("wqkv", [HL * D, 3 * E], F16, kind="ExternalInput")
        wo = nc.dram_tensor("wo", [HL * E, D], F16, kind="ExternalInput")
        bqk = nc.dram_tensor("bqk", [2, HL * E], F32, kind="ExternalInput")
        bvb = nc.dram_tensor("bvb", [P, HL * E], F32, kind="ExternalInput")
        ident = nc.dram_tensor("ident", [P, P], F16, kind="ExternalInput")
        dmsk = nc.dram_tensor("dmsk", [P, P], F32, kind="ExternalInput")
        outp = nc.dram_tensor("outp", [S, D], F16, kind="ExternalOutput")
        with tile.TileContext(nc) as tc:
            _trace_kernel(tc, xt, wqkv, wo, bqk, bvb, ident, dmsk, outp)
        nc.compile()
        _NC_CACHE["nc"] = nc
    return _NC_CACHE["nc"]


def kernel(normalized_resid_pre, W_Q, W_K, W_V, W_O, b_Q, b_K, b_V, b_O):
    x = np.asarray(normalized_resid_pre, np.float32)
    W_Q = np.asarray(W_Q, np.float32)
    W_K = np.asarray(W_K, np.float32)
    W_V = np.asarray(W_V, np.float32)
    W_O = np.asarray(W_O, np.float32)
    b_Q = np.asarray(b_Q, np.float32)
    b_K = np.asarray(b_K, np.float32)
    b_V = np.asarray(b_V, np.float32)
    b_O = np.asarray(b_O, np.float32)

    nc = _get_nc()

    # wqkv[h] = [W_Q[h] | W_K[h] | W_V[h]] along the E axis -> [HL*D, 3E]
    xt16 = [np.ascontiguousarray(x[b].T.astype(np.float16)) for b in range(B)]
    wqkv16 = []
    wo16 = []
    bqk32 = []
    bvb32 = []
    for g in range(NCORES // B):
        hs = slice(g * HL, (g + 1) * HL)
        wqkv16.append(
            np.ascontiguousarray(
                np.concatenate([W_Q[hs], W_K[hs], W_V[hs]], axis=2)
                .reshape(HL * D, 3 * E)
                .astype(np.float16)
            )
        )
        wo16.append(np.ascontiguousarray(W_O[hs].reshape(HL * E, D).astype(np.float16)))
        bqk32.append(
            np.ascontiguousarray(
                np.stack(
                    [
                        b_Q[hs].reshape(-1) * np.float32(INV_SQRT_E),
                        b_K[hs].reshape(-1),
                    ]
                )
            )
        )
        bvb32.append(
            np.ascontiguousarray(np.tile(b_V[hs].reshape(1, -1), (P, 1)))
        )

    ident16 = np.eye(P, dtype=np.float16)
    col = np.arange(P, dtype=np.float32)
    dmsk32 = np.ascontiguousarray(
        np.where(col[None, :] >= col[:, None], np.float32(0), np.float32(-1e9))
    )

    in_maps = []
    for core in range(NCORES):
        b, g = core // (NCORES // B), core % (NCORES // B)
        in_maps.append(
            {
                "xt": xt16[b],
                "wqkv": wqkv16[g],
                "wo": wo16[g],
                "bqk": bqk32[g],
                "bvb": bvb32[g],
                "ident": ident16,
                "dmsk": dmsk32,
            }
        )

    res = run_bass_kernel_spmd(nc, in_maps, core_ids=list(range(NCORES)))
    global LAST_RESULTS
    LAST_RESULTS = res
    out = np.zeros((B, S, D), np.float32)
    for core in range(NCORES):
        out[core // (NCORES // B)] += res.results[core]["outp"].astype(np.float32)
    out += b_O[None, None, :]
    return out
